# revision 1
# baseline (speedup 1.0000x reference)
"""Conditional BatchNorm1d (training mode) on 8 Trainium2 NeuronCores.

Strategy (data-parallel over N):
  - Shard x/labels along N across 8 cores (62500 rows each).
  - One-hot encodings of labels (both layouts) are precomputed host-side in
    bf16 and streamed in (~4 MB/core extra traffic; frees DVE/GPSIMD, whose
    16-partition one-hot builds dominated earlier profiles).
  - Pass 1 (per core): segment sums s1[c,f] = sum_{i: lab=c} x, s2 = sum x^2
    via one-hot matmul on the PE accumulating into PSUM. x is cast to bf16
    during the SWDGE DMA (halves pass-1 HBM traffic; the bf16 rounding error
    cancels statistically in the 31k-sample sums).
  - AllReduce the tiny [16,256] stats across the 8 cores.
  - Stats -> scale/shift [16,256] on-chip (mirrors the reference formulas).
  - Pass 2 (per core): per-row gather of scale/shift via transposed one-hot
    matmul in bf16 with hi/lo split (PSUM accumulation adds the halves, so
    the gather is fp32-exact to ~1e-7), then y = x*s + t on the DVE with
    quad-packed 3-D-AP ops.

Everything is hardcoded for the problem size: x [500000,128] f32,
labels [500000] int, gamma/beta [16,128] f32.
"""
import numpy as np

N_CORES = 8
N = 500000
F = 128
C = 16
EPS = 1e-5

SHARD = N // N_CORES         # 62500 real rows per core
P = 128                      # partitions per tile (16 DMA descriptors/transfer)
J = 20                       # subtiles per group (rows per partition)
GROUP = P * J                # 2560 rows per group
NG = 25                      # groups per core
ROWS = NG * GROUP            # 64000 padded rows per core
QUAD = 4                     # j-subtiles per psum tile / DVE op

_CACHE = {}


def _build():
    import concourse.bacc as bacc
    import concourse.bass as bass
    from concourse import mybir
    import concourse.tile as tile

    F32 = mybir.dt.float32
    BF16 = mybir.dt.bfloat16
    AF = mybir.ActivationFunctionType
    ALU = mybir.AluOpType

    nc = bacc.Bacc("TRN2", target_bir_lowering=False, debug=False,
                   num_devices=N_CORES)
    x = nc.dram_tensor("x", [ROWS, F], F32, kind="ExternalInput").ap()
    xb = nc.dram_tensor("xb", [ROWS, F], BF16, kind="ExternalInput").ap()
    h1 = nc.dram_tensor("h1", [ROWS, C], BF16, kind="ExternalInput").ap()
    ht = nc.dram_tensor("ht", [C, ROWS], BF16, kind="ExternalInput").ap()
    gamma = nc.dram_tensor("gamma", [C, F], F32, kind="ExternalInput").ap()
    beta = nc.dram_tensor("beta", [C, F], F32, kind="ExternalInput").ap()
    invn = nc.dram_tensor("invn", [C, 1], F32, kind="ExternalInput").ap()
    y = nc.dram_tensor("y", [ROWS, F], F32, kind="ExternalOutput").ap()

    with tile.TileContext(nc) as tc:
        with (
            tc.tile_pool(name="const", bufs=1) as const,
            tc.tile_pool(name="small", bufs=1) as small,
            tc.tile_pool(name="dram", bufs=1, space="DRAM") as dram,
            tc.tile_pool(name="psacc", bufs=1, space="PSUM") as psacc,
        ):
            # ---- constants ----
            gamma_sb = const.tile([C, F], F32)
            nc.sync.dma_start(out=gamma_sb[:], in_=gamma)
            beta_sb = const.tile([C, F], F32)
            nc.sync.dma_start(out=beta_sb[:], in_=beta)
            invn_sb = const.tile([C, 1], F32)
            nc.sync.dma_start(out=invn_sb[:], in_=invn)
            eps_sb = const.tile([C, 1], F32)
            nc.vector.memset(eps_sb[:], EPS)

            # ================= PASS 1: local stats =================
            psum_s12 = psacc.tile([C, 2 * F], F32)
            with tc.tile_pool(name="p1", bufs=4) as p1:
                for g in range(NG):
                    base = g * GROUP
                    # p-major: partition p holds rows [base+J*p, base+J*(p+1))
                    x_p = bass.AP(tensor=xb.tensor, offset=base * F,
                                  ap=[[J * F, P], [1, J * F]])
                    # xc = [x (J*F) | x^2 (J*F)]: both halves contiguous;
                    # matmul rhs reads [x_j | xsq_j] via a 2-D free AP.
                    xc = p1.tile([P, 2, J * F], BF16)
                    nc.sync.dma_start(out=xc[:, 0, :].opt(), in_=x_p.opt())
                    if g % 2 == 0:
                        nc.scalar.activation(out=xc[:, 1, :].opt(),
                                             in_=xc[:, 0, :].opt(), func=AF.Square)
                    else:
                        nc.vector.tensor_tensor(out=xc[:, 1, :].opt(),
                                                in0=xc[:, 0, :].opt(),
                                                in1=xc[:, 0, :].opt(),
                                                op=ALU.mult)
                    # one-hot H [125, 20, 16] (host-precomputed, contiguous)
                    h_p = bass.AP(tensor=h1.tensor, offset=base * C,
                                  ap=[[J * C, P], [1, J * C]])
                    H = p1.tile([P, J, C], BF16, tag="H")
                    nc.sync.dma_start(out=H[:].opt(), in_=h_p.opt())

                    xc0 = xc[:].opt()
                    for j in range(J):
                        rhs_j = bass.AP(tensor=xc.tensor,
                                        offset=xc0.offset + j * F,
                                        ap=[xc0.ap[0], [J * F, 2], [1, F]])
                        nc.tensor.matmul(
                            out=psum_s12[:],
                            lhsT=H[:, j, :],
                            rhs=rhs_j,
                            start=(g == 0 and j == 0),
                            stop=(g == NG - 1 and j == J - 1),
                        )

            # ================= AllReduce stats =================
            stats_sb = small.tile([C, 2 * F], F32)
            nc.vector.tensor_copy(out=stats_sb[:], in_=psum_s12[:])
            cc_in = dram.tile([C, 2 * F], F32)
            cc_out = dram.tile([C, 2 * F], F32)
            nc.scalar.dma_start(out=cc_in[:], in_=stats_sb[:])
            nc.gpsimd.collective_compute(
                "AllReduce",
                mybir.AluOpType.add,
                replica_groups=[list(range(N_CORES))],
                ins=[cc_in.opt()],
                outs=[cc_out.opt()],
            )
            stats_all = small.tile([C, 2 * F], F32)
            nc.scalar.dma_start(out=stats_all[:], in_=cc_out[:])

            # ---- stats -> scale/shift (mirrors reference formulas) ----
            mean = small.tile([C, F], F32)
            nc.vector.tensor_scalar(out=mean[:], in0=stats_all[:, 0:F],
                                    scalar1=invn_sb[:], scalar2=None, op0=ALU.mult)
            ex2 = small.tile([C, F], F32)
            nc.vector.tensor_scalar(out=ex2[:], in0=stats_all[:, F:2 * F],
                                    scalar1=invn_sb[:], scalar2=None, op0=ALU.mult)
            var = small.tile([C, F], F32)
            nc.vector.tensor_tensor(out=var[:], in0=mean[:], in1=mean[:], op=ALU.mult)
            nc.vector.tensor_tensor(out=var[:], in0=ex2[:], in1=var[:], op=ALU.subtract)
            std = small.tile([C, F], F32)
            nc.scalar.activation(out=std[:], in_=var[:], func=AF.Sqrt, bias=eps_sb[:])
            istd = small.tile([C, F], F32)
            nc.vector.reciprocal(out=istd[:], in_=std[:])
            sc_sh = small.tile([C, 2 * F], F32)
            nc.vector.tensor_tensor(out=sc_sh[:, 0:F], in0=gamma_sb[:],
                                    in1=istd[:], op=ALU.mult)
            ms = small.tile([C, F], F32)
            nc.vector.tensor_tensor(out=ms[:], in0=mean[:], in1=sc_sh[:, 0:F],
                                    op=ALU.mult)
            nc.vector.tensor_tensor(out=sc_sh[:, F:2 * F], in0=beta_sb[:],
                                    in1=ms[:], op=ALU.subtract)
            # bf16 hi/lo split: hi + lo == sc_sh to ~1e-7 (PSUM adds them)
            sc_hi = small.tile([C, 2 * F], BF16)
            nc.vector.tensor_copy(out=sc_hi[:], in_=sc_sh[:])
            sc_lo = small.tile([C, 2 * F], BF16)
            nc.vector.tensor_tensor(out=sc_lo[:], in0=sc_sh[:], in1=sc_hi[:],
                                    op=ALU.subtract)

            # ================= PASS 2: apply =================
            # p-major x/y; ht columns are host-permuted to (g, j, p) order so
            # lhsT for subtile j is the contiguous slice ht[:, base+125j:...].
            with tc.tile_pool(name="p2", bufs=4) as p2, \
                 tc.tile_pool(name="p2y", bufs=3) as p2y, \
                 tc.tile_pool(name="p2t", bufs=4) as p2t, \
                 tc.tile_pool(name="ps2", bufs=3, space="PSUM") as ps2:
                for g in range(NG):
                    base = g * GROUP
                    x_p = bass.AP(tensor=x.tensor, offset=base * F,
                                  ap=[[J * F, P], [1, J * F]])
                    y_p = bass.AP(tensor=y.tensor, offset=base * F,
                                  ap=[[J * F, P], [1, J * F]])
                    x2_tile = p2.tile([P, J, F], F32)
                    nc.sync.dma_start(out=x2_tile[:].opt(), in_=x_p.opt())
                    ht_ap = bass.AP(tensor=ht.tensor, offset=base,
                                    ap=[[ROWS, C], [1, GROUP]])
                    H_T = p2.tile([C, GROUP], BF16, tag="HT")
                    nc.sync.dma_start(out=H_T[:].opt(), in_=ht_ap.opt())

                    y_tile = p2y.tile([P, J, F], F32)
                    for q in range(J // QUAD):
                        psum_ss = ps2.tile([P, QUAD, 2 * F], F32)  # 2 banks
                        for h in range(QUAD):
                            j = QUAD * q + h
                            lhsT_j = H_T[:, P * j:P * (j + 1)]
                            nc.tensor.matmul(out=psum_ss[:, h, :], lhsT=lhsT_j,
                                             rhs=sc_hi[:], start=True, stop=False)
                            nc.tensor.matmul(out=psum_ss[:, h, :], lhsT=lhsT_j,
                                             rhs=sc_lo[:], start=False, stop=True)
                        j0 = QUAD * q
                        tmp = p2t.tile([P, QUAD, F], F32)
                        nc.vector.tensor_tensor(out=tmp[:],
                                                in0=x2_tile[:, j0:j0 + QUAD, :],
                                                in1=psum_ss[:, :, 0:F],
                                                op=ALU.mult)
                        nc.vector.tensor_tensor(out=y_tile[:, j0:j0 + QUAD, :],
                                                in0=tmp[:],
                                                in1=psum_ss[:, :, F:2 * F],
                                                op=ALU.add)
                    nc.scalar.dma_start(out=y_p.opt(), in_=y_tile[:].opt())
    nc.finalize()
    return nc


def _get_nc():
    if "nc" not in _CACHE:
        _CACHE["nc"] = _build()
    return _CACHE["nc"]


def _prep_host(labels_np):
    import ml_dtypes
    BF = ml_dtypes.bfloat16
    lab = labels_np.astype(np.int64)
    counts = np.maximum(np.bincount(lab, minlength=C), 1).astype(np.float64)
    invn = (1.0 / counts).astype(np.float32).reshape(C, 1)
    eye = np.eye(C, dtype=BF)
    h1_all, ht_all = [], []
    for k in range(N_CORES):
        lab_pad = np.full(ROWS, -1, dtype=np.int64)
        lab_pad[:SHARD] = lab[k * SHARD:(k + 1) * SHARD]
        h1 = np.zeros((ROWS, C), dtype=BF)
        h1[:SHARD] = eye[lab_pad[:SHARD]]
        h1_all.append(h1)
        # ht columns in (g, j, p) order: col g*GROUP+P*j+p holds onehot of
        # padded row g*GROUP + J*p + j (zero for pad rows).
        shard = lab_pad.reshape(NG, P, J)                        # (g, p, j)
        perm = shard.transpose(0, 2, 1).reshape(-1)              # (g, j, p)
        onehot_t = (perm[None, :] == np.arange(C)[:, None])
        ht_all.append(onehot_t.astype(BF))
    return h1_all, ht_all, invn


def kernel(x, labels, gamma, beta):
    from concourse.bass_utils import run_bass_kernel_spmd

    x = np.ascontiguousarray(np.asarray(x, dtype=np.float32))
    labels_np = np.asarray(labels)
    gamma = np.ascontiguousarray(np.asarray(gamma, dtype=np.float32))
    beta = np.ascontiguousarray(np.asarray(beta, dtype=np.float32))

    h1_all, ht_all, invn = _prep_host(labels_np)
    import ml_dtypes
    xb = x.astype(ml_dtypes.bfloat16)

    nc = _get_nc()
    in_maps = []
    for k in range(N_CORES):
        sl = slice(k * SHARD, (k + 1) * SHARD)
        x_pad = np.zeros((ROWS, F), dtype=np.float32)
        x_pad[:SHARD] = x[sl]
        xb_pad = np.zeros((ROWS, F), dtype=ml_dtypes.bfloat16)
        xb_pad[:SHARD] = xb[sl]
        in_maps.append({
            "x": x_pad,
            "xb": xb_pad,
            "h1": h1_all[k],
            "ht": ht_all[k],
            "gamma": gamma,
            "beta": beta,
            "invn": invn,
        })
    res = run_bass_kernel_spmd(nc, in_maps, core_ids=list(range(N_CORES)),
                               **_CACHE.get("run_kwargs", {}))
    _CACHE["last_results"] = res
    y = np.concatenate([res.results[k]["y"][:SHARD] for k in range(N_CORES)],
                       axis=0)
    return y



# revision 6
# speedup vs baseline: 1.6804x; 1.6804x over previous
"""Conditional BatchNorm1d (training mode) on 8 Trainium2 NeuronCores.

Strategy (data-parallel over N, class-slot layout):
  - Host groups rows by label and splits each class evenly across the 8
    cores. Each core receives x F-major (features on partitions) as
    xt [128, 16*4096] fp16: class c occupies the fixed column slot
    [c*4096, (c+1)*4096), zero-padded (slot capacity 4096 covers
    count_c <= 32768 globally, ~9 sigma for the uniform label fill).
  - With labels encoded purely in the layout, the per-row scale/shift
    gather disappears: scale[c]/shift[c] are per-partition [128,1]
    scalars for the whole slot.
  - Pass 1 (per slot): s1 = DVE free-dim reduce, s2 = ScalarE
    Square-activation with accum_out. fp32 accumulation.
  - AllReduce the [128, 32] (s1|s2) stats across the 8 cores.
  - Stats -> scale/shift [128,16] on-chip (mirrors reference formulas).
  - Pass 2 (per slot): y = x*scale_c + shift_c as ONE instruction per
    slot (DVE tensor_scalar / ScalarE activation(Identity, scale, bias)
    alternating), fp16 out. x stays resident in SBUF between passes, so
    HBM traffic is ~17 MB in + ~17 MB out per core. TensorE unused.
  - fp16 end-to-end rel_norm vs fp32 reference: ~2.3e-4.

Everything is hardcoded for the problem size: x [500000,128] f32,
labels [500000] int, gamma/beta [16,128] f32.
"""
import numpy as np

N_CORES = 8
N = 500000
F = 128
C = 16
EPS = 1e-5

SLOT = 4096                  # columns per class slot (per core)
COLS = C * SLOT              # 65536 columns per core

_CACHE = {}


def _build():
    import concourse.bacc as bacc
    import concourse.bass as bass
    from concourse import mybir
    import concourse.tile as tile

    F32 = mybir.dt.float32
    F16 = mybir.dt.float16
    AF = mybir.ActivationFunctionType
    ALU = mybir.AluOpType

    nc = bacc.Bacc("TRN2", target_bir_lowering=False, debug=False,
                   num_devices=N_CORES)
    xt = nc.dram_tensor("xt", [F, COLS], F16, kind="ExternalInput").ap()
    gt = nc.dram_tensor("gt", [F, C], F32, kind="ExternalInput").ap()
    bt = nc.dram_tensor("bt", [F, C], F32, kind="ExternalInput").ap()
    invn = nc.dram_tensor("invn", [F, C], F32, kind="ExternalInput").ap()
    y = nc.dram_tensor("y", [F, COLS], F16, kind="ExternalOutput").ap()

    with tile.TileContext(nc) as tc:
        with (
            tc.tile_pool(name="const", bufs=1) as const,
            tc.tile_pool(name="xres", bufs=C) as xres,
            tc.tile_pool(name="sq", bufs=2) as sqp,
            tc.tile_pool(name="yst", bufs=4) as yp,
            tc.tile_pool(name="dram", bufs=1, space="DRAM") as dram,
        ):
            gt_sb = const.tile([F, C], F32)
            nc.sync.dma_start(out=gt_sb[:], in_=gt)
            bt_sb = const.tile([F, C], F32)
            nc.sync.dma_start(out=bt_sb[:], in_=bt)
            invn_sb = const.tile([F, C], F32)
            nc.sync.dma_start(out=invn_sb[:], in_=invn)
            eps_sb = const.tile([F, 1], F32)
            nc.vector.memset(eps_sb[:], EPS)

            # ============ PASS 1: local stats (s1 | s2) ============
            # All input DMAs emitted first so the Act-queue DMAs enqueue
            # ahead of the squares in the Act instruction stream.
            stats_sb = const.tile([F, 2 * C], F32)
            xs = []
            for c in range(C):
                x_c = xres.tile([F, SLOT], F16, tag="x")
                src = bass.AP(tensor=xt.tensor, offset=c * SLOT,
                              ap=[[COLS, F], [1, SLOT]])
                eng = nc.sync if c % 2 == 0 else nc.scalar
                eng.dma_start(out=x_c[:], in_=src)
                xs.append(x_c)
            for c in range(C):
                nc.vector.tensor_reduce(out=stats_sb[:, c:c + 1],
                                        in_=xs[c][:],
                                        axis=mybir.AxisListType.X, op=ALU.add)
                sq = sqp.tile([F, SLOT], F16, tag="sq")
                nc.scalar.activation(out=sq[:], in_=xs[c][:], func=AF.Square,
                                     accum_out=stats_sb[:, C + c:C + c + 1])

            # ============ AllReduce stats ============
            cc_in = dram.tile([F, 2 * C], F32)
            cc_out = dram.tile([F, 2 * C], F32)
            nc.sync.dma_start(out=cc_in[:], in_=stats_sb[:])
            nc.gpsimd.collective_compute(
                "AllReduce",
                mybir.AluOpType.add,
                replica_groups=[list(range(N_CORES))],
                ins=[cc_in.opt()],
                outs=[cc_out.opt()],
            )
            gstats = const.tile([F, 2 * C], F32)
            nc.scalar.dma_start(out=gstats[:], in_=cc_out[:])

            # ---- stats -> scale/shift (mirrors reference formulas) ----
            mean = const.tile([F, C], F32)
            nc.vector.tensor_tensor(out=mean[:], in0=gstats[:, 0:C],
                                    in1=invn_sb[:], op=ALU.mult)
            ex2 = const.tile([F, C], F32)
            nc.vector.tensor_tensor(out=ex2[:], in0=gstats[:, C:2 * C],
                                    in1=invn_sb[:], op=ALU.mult)
            var = const.tile([F, C], F32)
            nc.vector.tensor_tensor(out=var[:], in0=mean[:], in1=mean[:],
                                    op=ALU.mult)
            nc.vector.tensor_tensor(out=var[:], in0=ex2[:], in1=var[:],
                                    op=ALU.subtract)
            std = const.tile([F, C], F32)
            nc.scalar.activation(out=std[:], in_=var[:], func=AF.Sqrt,
                                 bias=eps_sb[:])
            istd = const.tile([F, C], F32)
            nc.vector.reciprocal(out=istd[:], in_=std[:])
            scale = const.tile([F, C], F32)
            nc.vector.tensor_tensor(out=scale[:], in0=gt_sb[:], in1=istd[:],
                                    op=ALU.mult)
            msc = const.tile([F, C], F32)
            nc.vector.tensor_tensor(out=msc[:], in0=mean[:], in1=scale[:],
                                    op=ALU.mult)
            shift = const.tile([F, C], F32)
            nc.vector.tensor_tensor(out=shift[:], in0=bt_sb[:], in1=msc[:],
                                    op=ALU.subtract)

            # ============ PASS 2: y = x*scale_c + shift_c ============
            for c in range(C):
                yt = yp.tile([F, SLOT], F16, tag="y")
                if c % 2 == 0:
                    nc.vector.tensor_scalar(out=yt[:], in0=xs[c][:],
                                            scalar1=scale[:, c:c + 1],
                                            scalar2=shift[:, c:c + 1],
                                            op0=ALU.mult, op1=ALU.add)
                else:
                    nc.scalar.activation(out=yt[:], in_=xs[c][:],
                                         func=AF.Identity,
                                         bias=shift[:, c:c + 1],
                                         scale=scale[:, c:c + 1])
                dst = bass.AP(tensor=y.tensor, offset=c * SLOT,
                              ap=[[COLS, F], [1, SLOT]])
                eng = nc.sync if c % 2 == 0 else nc.scalar
                eng.dma_start(out=dst, in_=yt[:])
    nc.finalize()
    return nc


def _get_nc():
    if "nc" not in _CACHE:
        _CACHE["nc"] = _build()
    return _CACHE["nc"]


def _numpy_fallback(x, labels, gamma, beta):
    counts = np.maximum(np.bincount(labels, minlength=C), 1).astype(np.float32)
    s1 = np.zeros((C, F), np.float32)
    s2 = np.zeros((C, F), np.float32)
    for c in range(C):
        m = labels == c
        s1[c] = x[m].sum(0)
        s2[c] = (x[m] * x[m]).sum(0)
    mean = s1 / counts[:, None]
    var = s2 / counts[:, None] - mean * mean
    istd = 1.0 / np.sqrt(var + EPS)
    scale = gamma * istd
    shift = beta - mean * scale
    return x * scale[labels] + shift[labels]


def kernel(x, labels, gamma, beta):
    from concourse.bass_utils import run_bass_kernel_spmd

    x = np.ascontiguousarray(np.asarray(x, dtype=np.float32))
    labels_np = np.asarray(labels).astype(np.int64)
    gamma = np.ascontiguousarray(np.asarray(gamma, dtype=np.float32))
    beta = np.ascontiguousarray(np.asarray(beta, dtype=np.float32))

    counts = np.bincount(labels_np, minlength=C)
    if int(counts.max()) > N_CORES * SLOT:
        return _numpy_fallback(x, labels_np, gamma, beta)

    # group rows by label; split each class evenly across cores
    order = np.argsort(labels_np, kind="stable")
    starts = np.concatenate([[0], np.cumsum(counts)])
    chunks = [np.array_split(order[starts[c]:starts[c + 1]], N_CORES)
              for c in range(C)]

    invn = (1.0 / np.maximum(counts, 1)).astype(np.float32)
    invn_b = np.ascontiguousarray(np.broadcast_to(invn, (F, C)))
    gt = np.ascontiguousarray(gamma.T)
    bt = np.ascontiguousarray(beta.T)

    xh = x.astype(np.float16)
    in_maps = []
    for k in range(N_CORES):
        xt_k = np.zeros((F, COLS), dtype=np.float16)
        for c in range(C):
            rows = chunks[c][k]
            xt_k[:, c * SLOT:c * SLOT + len(rows)] = xh[rows].T
        in_maps.append({"xt": xt_k, "gt": gt, "bt": bt, "invn": invn_b})

    nc = _get_nc()
    res = run_bass_kernel_spmd(nc, in_maps, core_ids=list(range(N_CORES)),
                               **_CACHE.get("run_kwargs", {}))
    _CACHE["last_results"] = res

    y = np.empty((N, F), dtype=np.float32)
    for k in range(N_CORES):
        yk = res.results[k]["y"]
        for c in range(C):
            rows = chunks[c][k]
            y[rows] = yk[:, c * SLOT:c * SLOT + len(rows)].T
    return y
